# revision 17
# baseline (speedup 1.0000x reference)
"""Trainium2 Bass kernel: 2-layer LSTM (H=64, D=8, T=256) + FC head, batch 8192.

Strategy (pure data parallel, 8 cores x 1024 batch):
  - Stage s computes layer0 timestep s and layer1 timestep s-1 simultaneously,
    with all per-gate tensors stacked [layer0(64p); layer1(64p)] on 128
    partitions.  The stacked hidden state h_stack = [h0_s; h1_{s-1}] is exactly
    the rhs the layer1 matmul of the next stage needs (K=128).
  - Gates are computed as gatesT [gate, batch] via PE matmuls with the small
    weights stationary; batch is the moving free dim (fp16 operands, fp32 PSUM).
    Each gate bank takes an l0 (K=72, cols 0:64) + l1 (K=128, cols 64:128)
    matmul pair that dual-issues on distinct PE column groups.
  - Sigmoid/tanh on the ACT engine (the bottleneck: 10 ACTIVATEs x FD=512
    per stage = ~5.84us/stage, ~100% issue-saturated), cell update on DVE.
    fp16 everywhere off-PSUM keeps every DVE tensor_tensor in 2x mode, which
    shortens the recurrence loop below the ACT throughput floor.
  - x is pre-transposed and pre-cast ON THE HOST to the [t%16*8+d
    (partitions), tci*BC+b] fp16 layout, so the kernel just DMAs it in
    (chunk 0 first) and per-stage x slices are DMA-staged into the xh tile.
  - Batch is split into 2 subtiles of 512 that pipeline through the engines.
  - The 257 stages are split across two sequential TileContexts: event-sem
    values saturate (~4096) in a single fully-unrolled context; each context
    exit drains + resets semaphores.  LSTM state crosses the boundary in
    persistent (non-pool) SBUF tensors.
"""

import numpy as np
import ml_dtypes

import concourse.bass as bass
import concourse.bacc as bacc
import concourse.mybir as mybir
import concourse.tile as tile
from concourse.bass_utils import run_bass_kernel_spmd

F32 = mybir.dt.float32
F16 = mybir.dt.float16
AF = mybir.ActivationFunctionType

H = 64
D = 8
T_FULL = 256
B_TOTAL = 8192
N_CORES = 8
BC = B_TOTAL // N_CORES  # 1024 per core
NSUB = 2
BSUB = BC // NSUB  # 512

GATES = "ifgo"  # PyTorch order; gate j occupies rows j*64:(j+1)*64 of 4H


class _Consts:
    pass


def _emit_stage(nc, spool, gpool, cst, st, s, u, n_stage, boundary_out=None):
    """Emit one (stage, subtile) unit.  boundary_out: dict of raw APs to write
    state into (instead of pool tiles) when this is the last stage of a context."""
    bb = 4 if s == 0 else 0  # stage-0 biases zero the l1 half
    P = {}
    for g in GATES:
        P[g] = gpool.tile([128, BSUB], F32, name=f"P_{g}_u{u}", tag=f"P_{g}_u{u}")
    # gate order i,g first (unblocks the t_ig chain); l0/l1 pairs dual-issue
    for g in "igfo":
        j = GATES.index(g)
        nc.tensor.matmul(
            P[g][0:64, :],
            cst.w0[:, j * 64 : j * 64 + 64],
            st["xh"][u],
            start=True,
            stop=True,
        )
        nc.tensor.matmul(
            P[g][64:128, :],
            cst.w1[:, j * 64 : j * 64 + 64],
            st["h"][u],
            start=True,
            stop=True,
        )
    S_i = spool.tile([128, BSUB], F16, name=f"S_i_u{u}", tag=f"S_i_u{u}")
    nc.scalar.activation(S_i, P["i"], AF.Sigmoid, bias=cst.bias[:, bb + 0 : bb + 1])
    T_g = spool.tile([128, BSUB], F16, name=f"T_g_u{u}", tag=f"T_g_u{u}")
    nc.scalar.activation(T_g, P["g"], AF.Tanh, bias=cst.bias[:, bb + 2 : bb + 3])
    S_f = spool.tile([128, BSUB], F16, name=f"S_f_u{u}", tag=f"S_f_u{u}")
    nc.scalar.activation(S_f, P["f"], AF.Sigmoid, bias=cst.bias[:, bb + 1 : bb + 2])
    S_o = spool.tile([128, BSUB], F16, name=f"S_o_u{u}", tag=f"S_o_u{u}")
    nc.scalar.activation(S_o, P["o"], AF.Sigmoid, bias=cst.bias[:, bb + 3 : bb + 4])

    t_ig = spool.tile([128, BSUB], F16, name=f"t_ig_u{u}", tag=f"t_ig_u{u}")
    nc.vector.tensor_mul(t_ig, S_i, T_g)
    t_fc = spool.tile([128, BSUB], F16, name=f"t_fc_u{u}", tag=f"t_fc_u{u}")
    nc.vector.tensor_mul(t_fc, S_f, st["c"][u])
    if boundary_out is not None:
        c_new = boundary_out["c"][u]
    else:
        c_new = spool.tile([128, BSUB], F16, name=f"cst_u{u}", tag=f"cst_u{u}")
    nc.vector.tensor_add(c_new, t_fc, t_ig)
    T_c = spool.tile([128, BSUB], F16, name=f"T_c_u{u}", tag=f"T_c_u{u}")
    nc.scalar.activation(T_c, c_new, AF.Tanh)
    if boundary_out is not None:
        h_new = boundary_out["h"][u]
    else:
        h_new = spool.tile([128, BSUB], F16, name=f"hst_u{u}", tag=f"hst_u{u}")
    nc.vector.tensor_mul(h_new, S_o, T_c)

    if s < n_stage - 1:
        tn = s + 1
        if boundary_out is not None:
            xh_n = boundary_out["xh"][u]
        else:
            xh_n = spool.tile([72, BSUB], F16, name=f"xh_u{u}", tag=f"xh_u{u}")
        nc.vector.tensor_copy(xh_n[0:64, :], h_new[0:64, :])
        nc.sync.dma_start(
            xh_n[64:72, :],
            cst.xT[
                (tn % 16) * 8 : (tn % 16) * 8 + 8,
                (tn // 16) * BC + u * BSUB : (tn // 16) * BC + (u + 1) * BSUB,
            ],
        )
        st["xh"][u] = xh_n
    st["h"][u] = h_new
    st["c"][u] = c_new


def _emit_fc(nc, spool, gpool, cst, st, u):
    """final head: logits = h1_{T-1} @ Wfc.T + bfc ; sigmoid.  Emitted AFTER
    both subtiles' last stage so sigma_out(u0) doesn't block u1's activations
    in the strict-FIFO ACT queue while waiting on h/P_fc."""
    P_fc = gpool.tile([1, BSUB], F32, name=f"P_fc_u{u}", tag=f"P_i_u{u}")
    nc.tensor.matmul(P_fc, cst.wfc, st["h"][u], start=True, stop=True)
    S_out = spool.tile([1, BSUB], F32, name=f"S_out_u{u}", tag=f"S_out_u{u}")
    nc.scalar.activation(S_out, P_fc, AF.Sigmoid, bias=cst.bias[0:1, 8:9])
    nc.sync.dma_start(cst.out_d[u : u + 1, :], S_out)


def _build_module(t_steps=T_FULL):
    assert t_steps % 16 == 0
    n_stage = t_steps + 1
    # split stages into sem-safe chunks (PE completion sem must stay < ~4096;
    # PE increments ~16/stage)
    n_ctx0 = min(128, n_stage - 1)
    nc = bacc.Bacc("TRN2", target_bir_lowering=False, debug=False, enable_asserts=False)
    xt_d = nc.dram_tensor("xt", [128, (t_steps // 16 + 1) * BC], F16, kind="ExternalInput").ap()
    w0_d = nc.dram_tensor("w0", [72, 256], F16, kind="ExternalInput").ap()
    w1_d = nc.dram_tensor("w1", [128, 256], F16, kind="ExternalInput").ap()
    wfc_d = nc.dram_tensor("wfc", [128, 1], F16, kind="ExternalInput").ap()
    bias_d = nc.dram_tensor("biases", [128, 16], F32, kind="ExternalInput").ap()
    out_d = nc.dram_tensor("out", [NSUB, BSUB], F32, kind="ExternalOutput").ap()

    n_tc = t_steps // 16
    cst = _Consts()
    cst.w0 = nc.alloc_sbuf_tensor("w0_sb", [72, 256], F16).ap()
    cst.w1 = nc.alloc_sbuf_tensor("w1_sb", [128, 256], F16).ap()
    cst.wfc = nc.alloc_sbuf_tensor("wfc_sb", [128, 1], F16).ap()
    cst.bias = nc.alloc_sbuf_tensor("bias_sb", [128, 16], F32).ap()
    cst.xT = nc.alloc_sbuf_tensor("xT_sb", [128, (n_tc + 1) * BC], F16).ap()
    cst.out_d = out_d
    # boundary state (crosses the TileContext barrier)
    hb = [nc.alloc_sbuf_tensor(f"hb_u{u}", [128, BSUB], F16).ap() for u in range(NSUB)]
    cb = [nc.alloc_sbuf_tensor(f"cb_u{u}", [128, BSUB], F16).ap() for u in range(NSUB)]
    xb = [nc.alloc_sbuf_tensor(f"xb_u{u}", [72, BSUB], F16).ap() for u in range(NSUB)]
    bound = {"h": hb, "c": cb, "xh": xb}

    # ---- context 0: consts, transpose, stages 0..n_ctx0-1 ----
    with tile.TileContext(nc) as tc:
        with tc.sbuf_pool(name="state0", bufs=2) as spool:
            # sync-queue order is stage-0's critical path: x chunk 0 first,
            # then the matmul weights, then the xh inits that read chunk 0 —
            # the first matmul waits on max(w1, xh), so balance both chains.
            nc.sync.dma_start(cst.xT[:, 0:BC], xt_d[:, 0:BC])
            nc.sync.dma_start(cst.w0, w0_d)
            nc.sync.dma_start(cst.w1, w1_d)

            with tc.psum_pool(name="pg0", bufs=1) as gpool:
                st = {"h": [None] * NSUB, "c": [None] * NSUB, "xh": [None] * NSUB}
                for u in range(NSUB):
                    h0t = spool.tile([128, BSUB], F16, name=f"hst_u{u}", tag=f"hst_u{u}")
                    nc.gpsimd.memset(h0t, 0.0)
                    c0t = spool.tile([128, BSUB], F16, name=f"cst_u{u}", tag=f"cst_u{u}")
                    nc.gpsimd.memset(c0t, 0.0)
                    xht = spool.tile([72, BSUB], F16, name=f"xh_u{u}", tag=f"xh_u{u}")
                    nc.gpsimd.memset(xht[0:64, :], 0.0)
                    nc.sync.dma_start(xht[64:72, :], cst.xT[0:8, u * BSUB : (u + 1) * BSUB])
                    st["h"][u] = h0t
                    st["c"][u] = c0t
                    st["xh"][u] = xht
                nc.sync.dma_start(cst.bias, bias_d)
                nc.sync.dma_start(cst.wfc, wfc_d)
                # bulk xT loads AFTER the state memsets so the memsets are not
                # queued behind 16 DMA triggers on the gpsimd ring; chunk-
                # granular writes keep stage-side xT readers from RAW-waiting
                for tci in range(1, n_tc + 1):
                    nc.gpsimd.dma_start(
                        cst.xT[:, tci * BC : (tci + 1) * BC],
                        xt_d[:, tci * BC : (tci + 1) * BC],
                    )
                for s in range(n_ctx0):
                    is_bound = s == n_ctx0 - 1
                    for u in range(NSUB):
                        _emit_stage(
                            nc, spool, gpool, cst, st, s, u, n_stage,
                            boundary_out=bound if is_bound else None,
                        )

    # ---- context 1: stages n_ctx0..n_stage-1 + fc ----
    with tile.TileContext(nc) as tc:
        with tc.sbuf_pool(name="state1", bufs=2) as spool:
            with tc.psum_pool(name="pg1", bufs=1) as gpool:
                st = {"h": list(hb), "c": list(cb), "xh": list(xb)}
                for s in range(n_ctx0, n_stage):
                    for u in range(NSUB):
                        _emit_stage(nc, spool, gpool, cst, st, s, u, n_stage)
                for u in range(NSUB):
                    _emit_fc(nc, spool, gpool, cst, st, u)

    nc.compile()
    return nc


def _prep_weights(Wih0, Whh0, bih0, bhh0, Wih1, Whh1, bih1, bhh1, Wfc, bfc):
    bf = np.float16
    w0 = np.concatenate([Whh0.T, Wih0.T], axis=0).astype(bf)  # [72, 256]
    w1 = np.concatenate([Wih1.T, Whh1.T], axis=0).astype(bf)  # [128, 256]
    wfc = np.concatenate(
        [np.zeros((64, 1), np.float32), Wfc.reshape(1, 64).T], axis=0
    ).astype(bf)  # [128, 1]
    b0 = (bih0 + bhh0).astype(np.float32)
    b1 = (bih1 + bhh1).astype(np.float32)
    biases = np.zeros((128, 16), np.float32)
    for j in range(4):
        biases[0:64, j] = b0[j * 64 : (j + 1) * 64]
        biases[64:128, j] = b1[j * 64 : (j + 1) * 64]
        biases[0:64, 4 + j] = b0[j * 64 : (j + 1) * 64]  # stage-0: l1 half stays 0
    biases[0, 8] = np.float32(bfc[0])
    return w0, w1, wfc, biases


_MODULE_CACHE = {}


def _get_module(t_steps=T_FULL):
    if t_steps not in _MODULE_CACHE:
        _MODULE_CACHE[t_steps] = _build_module(t_steps)
    return _MODULE_CACHE[t_steps]


def _run(inputs, trace=False, **spmd_kwargs):
    x = np.asarray(inputs["x"], np.float32)
    w0, w1, wfc, biases = _prep_weights(
        np.asarray(inputs["Wih0"], np.float32),
        np.asarray(inputs["Whh0"], np.float32),
        np.asarray(inputs["bih0"], np.float32),
        np.asarray(inputs["bhh0"], np.float32),
        np.asarray(inputs["Wih1"], np.float32),
        np.asarray(inputs["Whh1"], np.float32),
        np.asarray(inputs["bih1"], np.float32),
        np.asarray(inputs["bhh1"], np.float32),
        np.asarray(inputs["Wfc"], np.float32),
        np.asarray(inputs["bfc"], np.float32),
    )
    nc = _get_module(T_FULL)
    in_maps = []
    n_tc = T_FULL // 16
    pad = np.zeros((128, BC), np.float16)
    for c in range(N_CORES):
        shard = x[c * BC : (c + 1) * BC].reshape(BC, n_tc, 16, D)
        xt = shard.transpose(2, 3, 1, 0).reshape(128, n_tc * BC).astype(np.float16)
        xt = np.ascontiguousarray(np.concatenate([xt, pad], axis=1))
        in_maps.append(
            {"xt": xt, "w0": w0, "w1": w1, "wfc": wfc, "biases": biases}
        )
    res = run_bass_kernel_spmd(
        nc, in_maps, core_ids=list(range(N_CORES)), trace=trace, **spmd_kwargs
    )
    out = np.concatenate(
        [np.asarray(res.results[c]["out"]).reshape(BC, 1) for c in range(N_CORES)],
        axis=0,
    ).astype(np.float32)
    return out, res


def kernel(**inputs):
    out, _ = _run(inputs, trace=False)
    return out



# revision 19
# speedup vs baseline: 1.0019x; 1.0019x over previous
"""Trainium2 Bass kernel: 2-layer LSTM (H=64, D=8, T=256) + FC head, batch 8192.

Strategy (pure data parallel, 8 cores x 1024 batch):
  - Stage s computes layer0 timestep s and layer1 timestep s-1 simultaneously,
    with all per-gate tensors stacked [layer0(64p); layer1(64p)] on 128
    partitions.  The stacked hidden state h_stack = [h0_s; h1_{s-1}] is exactly
    the rhs the layer1 matmul of the next stage needs (K=128).
  - Gates are computed as gatesT [gate, batch] via PE matmuls with the small
    weights stationary; batch is the moving free dim (fp16 operands, fp32 PSUM).
    Each gate bank takes an l0 (K=72, cols 0:64) + l1 (K=128, cols 64:128)
    matmul pair that dual-issues on distinct PE column groups.
  - Sigmoid/tanh on the ACT engine (the bottleneck: 10 ACTIVATEs x FD=512
    per stage = ~5.84us/stage, ~100% issue-saturated), cell update on DVE.
    fp16 everywhere off-PSUM keeps every DVE tensor_tensor in 2x mode, which
    shortens the recurrence loop below the ACT throughput floor.
  - x is pre-transposed and pre-cast ON THE HOST to the [t%16*8+d
    (partitions), tci*BC+b] fp16 layout, so the kernel just DMAs it in
    (chunk 0 first) and per-stage x slices are DMA-staged into the xh tile.
  - Batch is split into 2 subtiles of 512 that pipeline through the engines.
  - The 257 stages are split across two sequential TileContexts: event-sem
    values saturate (~4096) in a single fully-unrolled context; each context
    exit drains + resets semaphores.  LSTM state crosses the boundary in
    persistent (non-pool) SBUF tensors.
"""

import numpy as np
import ml_dtypes

import concourse.bass as bass
import concourse.bacc as bacc
import concourse.mybir as mybir
import concourse.tile as tile
from concourse.bass_utils import run_bass_kernel_spmd

F32 = mybir.dt.float32
F16 = mybir.dt.float16
AF = mybir.ActivationFunctionType

H = 64
D = 8
T_FULL = 256
B_TOTAL = 8192
N_CORES = 8
BC = B_TOTAL // N_CORES  # 1024 per core
NSUB = 2
BSUB = BC // NSUB  # 512

GATES = "ifgo"  # PyTorch order; gate j occupies rows j*64:(j+1)*64 of 4H


class _Consts:
    pass


def _emit_stage(nc, spool, gpool, cst, st, s, u, n_stage, boundary_out=None):
    """Emit one (stage, subtile) unit.  boundary_out: dict of raw APs to write
    state into (instead of pool tiles) when this is the last stage of a context."""
    bb = 4 if s == 0 else 0  # stage-0 biases zero the l1 half
    P = {}
    for g in GATES:
        P[g] = gpool.tile([128, BSUB], F32, name=f"P_{g}_u{u}", tag=f"P_{g}_u{u}")
    # gate order i,g first (unblocks the t_ig chain); l0/l1 pairs dual-issue
    for g in "igfo":
        j = GATES.index(g)
        nc.tensor.matmul(
            P[g][0:64, :],
            cst.w0[:, j * 64 : j * 64 + 64],
            st["xh"][u],
            start=True,
            stop=True,
        )
        nc.tensor.matmul(
            P[g][64:128, :],
            cst.w1[:, j * 64 : j * 64 + 64],
            st["h"][u],
            start=True,
            stop=True,
        )
    S_i = spool.tile([128, BSUB], F16, name=f"S_i_u{u}", tag=f"S_i_u{u}")
    nc.scalar.activation(S_i, P["i"], AF.Sigmoid, bias=cst.bias[:, bb + 0 : bb + 1])
    T_g = spool.tile([128, BSUB], F16, name=f"T_g_u{u}", tag=f"T_g_u{u}")
    nc.scalar.activation(T_g, P["g"], AF.Tanh, bias=cst.bias[:, bb + 2 : bb + 3])
    S_f = spool.tile([128, BSUB], F16, name=f"S_f_u{u}", tag=f"S_f_u{u}")
    nc.scalar.activation(S_f, P["f"], AF.Sigmoid, bias=cst.bias[:, bb + 1 : bb + 2])
    S_o = spool.tile([128, BSUB], F16, name=f"S_o_u{u}", tag=f"S_o_u{u}")
    nc.scalar.activation(S_o, P["o"], AF.Sigmoid, bias=cst.bias[:, bb + 3 : bb + 4])

    t_ig = spool.tile([128, BSUB], F16, name=f"t_ig_u{u}", tag=f"t_ig_u{u}")
    nc.vector.tensor_mul(t_ig, S_i, T_g)
    t_fc = spool.tile([128, BSUB], F16, name=f"t_fc_u{u}", tag=f"t_fc_u{u}")
    nc.vector.tensor_mul(t_fc, S_f, st["c"][u])
    if boundary_out is not None:
        c_new = boundary_out["c"][u]
    else:
        c_new = spool.tile([128, BSUB], F16, name=f"cst_u{u}", tag=f"cst_u{u}")
    nc.vector.tensor_add(c_new, t_fc, t_ig)
    T_c = spool.tile([128, BSUB], F16, name=f"T_c_u{u}", tag=f"T_c_u{u}")
    nc.scalar.activation(T_c, c_new, AF.Tanh)
    if boundary_out is not None:
        h_new = boundary_out["h"][u]
    else:
        h_new = spool.tile([128, BSUB], F16, name=f"hst_u{u}", tag=f"hst_u{u}")
    nc.vector.tensor_mul(h_new, S_o, T_c)

    if s < n_stage - 1:
        tn = s + 1
        if boundary_out is not None:
            xh_n = boundary_out["xh"][u]
        else:
            xh_n = spool.tile([72, BSUB], F16, name=f"xh_u{u}", tag=f"xh_u{u}")
        nc.vector.tensor_copy(xh_n[0:64, :], h_new[0:64, :])
        nc.sync.dma_start(
            xh_n[64:72, :],
            cst.xT[
                (tn % 16) * 8 : (tn % 16) * 8 + 8,
                (tn // 16) * BC + u * BSUB : (tn // 16) * BC + (u + 1) * BSUB,
            ],
        )
        st["xh"][u] = xh_n
    st["h"][u] = h_new
    st["c"][u] = c_new


def _emit_fc(nc, spool, gpool, cst, st, u):
    """final head: logits = h1_{T-1} @ Wfc.T + bfc ; sigmoid.  Emitted AFTER
    both subtiles' last stage so sigma_out(u0) doesn't block u1's activations
    in the strict-FIFO ACT queue while waiting on h/P_fc."""
    P_fc = gpool.tile([1, BSUB], F32, name=f"P_fc_u{u}", tag=f"P_i_u{u}")
    nc.tensor.matmul(P_fc, cst.wfc, st["h"][u], start=True, stop=True)
    S_out = spool.tile([1, BSUB], F32, name=f"S_out_u{u}", tag=f"S_out_u{u}")
    nc.scalar.activation(S_out, P_fc, AF.Sigmoid, bias=cst.bias[0:1, 8:9])
    nc.sync.dma_start(cst.out_d[u : u + 1, :], S_out)


def _build_module(t_steps=T_FULL):
    assert t_steps % 16 == 0
    n_stage = t_steps + 1
    # split stages into sem-safe chunks (PE completion sem must stay < ~4096;
    # PE increments ~16/stage)
    n_ctx0 = min(128, n_stage - 1)
    nc = bacc.Bacc("TRN2", target_bir_lowering=False, debug=False, enable_asserts=False)
    xt_d = nc.dram_tensor("xt", [128, (t_steps // 16 + 1) * BC], F16, kind="ExternalInput").ap()
    xs0_d = nc.dram_tensor("xs0", [8, BC], F16, kind="ExternalInput").ap()
    w0_d = nc.dram_tensor("w0", [72, 256], F16, kind="ExternalInput").ap()
    w1_d = nc.dram_tensor("w1", [128, 256], F16, kind="ExternalInput").ap()
    wfc_d = nc.dram_tensor("wfc", [128, 1], F16, kind="ExternalInput").ap()
    bias_d = nc.dram_tensor("biases", [128, 16], F32, kind="ExternalInput").ap()
    out_d = nc.dram_tensor("out", [NSUB, BSUB], F32, kind="ExternalOutput").ap()

    n_tc = t_steps // 16
    cst = _Consts()
    cst.w0 = nc.alloc_sbuf_tensor("w0_sb", [72, 256], F16).ap()
    cst.w1 = nc.alloc_sbuf_tensor("w1_sb", [128, 256], F16).ap()
    cst.wfc = nc.alloc_sbuf_tensor("wfc_sb", [128, 1], F16).ap()
    cst.bias = nc.alloc_sbuf_tensor("bias_sb", [128, 16], F32).ap()
    cst.xT = nc.alloc_sbuf_tensor("xT_sb", [128, (n_tc + 1) * BC], F16).ap()
    cst.out_d = out_d
    # boundary state (crosses the TileContext barrier)
    hb = [nc.alloc_sbuf_tensor(f"hb_u{u}", [128, BSUB], F16).ap() for u in range(NSUB)]
    cb = [nc.alloc_sbuf_tensor(f"cb_u{u}", [128, BSUB], F16).ap() for u in range(NSUB)]
    xb = [nc.alloc_sbuf_tensor(f"xb_u{u}", [72, BSUB], F16).ap() for u in range(NSUB)]
    bound = {"h": hb, "c": cb, "xh": xb}

    # ---- context 0: consts, transpose, stages 0..n_ctx0-1 ----
    with tile.TileContext(nc) as tc:
        with tc.sbuf_pool(name="state0", bufs=2) as spool:
            # stage-0 critical path: xh inits come from a dedicated tiny
            # DRAM input (no chunk-0 dependency) and go FIRST on the sync
            # queue; weights ride the idle vector/scalar/gpsimd queues so
            # everything stage 0 needs lands concurrently.
            with tc.psum_pool(name="pg0", bufs=1) as gpool:
                st = {"h": [None] * NSUB, "c": [None] * NSUB, "xh": [None] * NSUB}
                for u in range(NSUB):
                    h0t = spool.tile([128, BSUB], F16, name=f"hst_u{u}", tag=f"hst_u{u}")
                    nc.gpsimd.memset(h0t, 0.0)
                    c0t = spool.tile([128, BSUB], F16, name=f"cst_u{u}", tag=f"cst_u{u}")
                    nc.gpsimd.memset(c0t, 0.0)
                    xht = spool.tile([72, BSUB], F16, name=f"xh_u{u}", tag=f"xh_u{u}")
                    nc.gpsimd.memset(xht[0:64, :], 0.0)
                    nc.sync.dma_start(xht[64:72, :], xs0_d[:, u * BSUB : (u + 1) * BSUB])
                    st["h"][u] = h0t
                    st["c"][u] = c0t
                    st["xh"][u] = xht
                nc.scalar.dma_start(cst.w0, w0_d)
                nc.scalar.dma_start(cst.w1, w1_d)
                nc.gpsimd.dma_start(cst.bias, bias_d)
                nc.sync.dma_start(cst.xT[:, 0:BC], xt_d[:, 0:BC])
                nc.sync.dma_start(cst.wfc, wfc_d)
                # bulk xT loads AFTER the state memsets so the memsets are not
                # queued behind 16 DMA triggers on the gpsimd ring; chunk-
                # granular writes keep stage-side xT readers from RAW-waiting
                for tci in range(1, n_tc + 1):
                    nc.gpsimd.dma_start(
                        cst.xT[:, tci * BC : (tci + 1) * BC],
                        xt_d[:, tci * BC : (tci + 1) * BC],
                    )
                for s in range(n_ctx0):
                    is_bound = s == n_ctx0 - 1
                    for u in range(NSUB):
                        _emit_stage(
                            nc, spool, gpool, cst, st, s, u, n_stage,
                            boundary_out=bound if is_bound else None,
                        )

    # ---- context 1: stages n_ctx0..n_stage-1 + fc ----
    with tile.TileContext(nc) as tc:
        with tc.sbuf_pool(name="state1", bufs=2) as spool:
            with tc.psum_pool(name="pg1", bufs=1) as gpool:
                st = {"h": list(hb), "c": list(cb), "xh": list(xb)}
                for s in range(n_ctx0, n_stage):
                    for u in range(NSUB):
                        _emit_stage(nc, spool, gpool, cst, st, s, u, n_stage)
                for u in range(NSUB):
                    _emit_fc(nc, spool, gpool, cst, st, u)

    nc.compile()
    return nc


def _prep_weights(Wih0, Whh0, bih0, bhh0, Wih1, Whh1, bih1, bhh1, Wfc, bfc):
    bf = np.float16
    w0 = np.concatenate([Whh0.T, Wih0.T], axis=0).astype(bf)  # [72, 256]
    w1 = np.concatenate([Wih1.T, Whh1.T], axis=0).astype(bf)  # [128, 256]
    wfc = np.concatenate(
        [np.zeros((64, 1), np.float32), Wfc.reshape(1, 64).T], axis=0
    ).astype(bf)  # [128, 1]
    b0 = (bih0 + bhh0).astype(np.float32)
    b1 = (bih1 + bhh1).astype(np.float32)
    biases = np.zeros((128, 16), np.float32)
    for j in range(4):
        biases[0:64, j] = b0[j * 64 : (j + 1) * 64]
        biases[64:128, j] = b1[j * 64 : (j + 1) * 64]
        biases[0:64, 4 + j] = b0[j * 64 : (j + 1) * 64]  # stage-0: l1 half stays 0
    biases[0, 8] = np.float32(bfc[0])
    return w0, w1, wfc, biases


_MODULE_CACHE = {}


def _get_module(t_steps=T_FULL):
    if t_steps not in _MODULE_CACHE:
        _MODULE_CACHE[t_steps] = _build_module(t_steps)
    return _MODULE_CACHE[t_steps]


def _run(inputs, trace=False, **spmd_kwargs):
    x = np.asarray(inputs["x"], np.float32)
    w0, w1, wfc, biases = _prep_weights(
        np.asarray(inputs["Wih0"], np.float32),
        np.asarray(inputs["Whh0"], np.float32),
        np.asarray(inputs["bih0"], np.float32),
        np.asarray(inputs["bhh0"], np.float32),
        np.asarray(inputs["Wih1"], np.float32),
        np.asarray(inputs["Whh1"], np.float32),
        np.asarray(inputs["bih1"], np.float32),
        np.asarray(inputs["bhh1"], np.float32),
        np.asarray(inputs["Wfc"], np.float32),
        np.asarray(inputs["bfc"], np.float32),
    )
    nc = _get_module(T_FULL)
    in_maps = []
    n_tc = T_FULL // 16
    pad = np.zeros((128, BC), np.float16)
    for c in range(N_CORES):
        shard = x[c * BC : (c + 1) * BC].reshape(BC, n_tc, 16, D)
        xt = shard.transpose(2, 3, 1, 0).reshape(128, n_tc * BC).astype(np.float16)
        xt = np.ascontiguousarray(np.concatenate([xt, pad], axis=1))
        in_maps.append(
            {"xt": xt, "xs0": np.ascontiguousarray(xt[0:8, 0:BC]),
             "w0": w0, "w1": w1, "wfc": wfc, "biases": biases}
        )
    res = run_bass_kernel_spmd(
        nc, in_maps, core_ids=list(range(N_CORES)), trace=trace, **spmd_kwargs
    )
    out = np.concatenate(
        [np.asarray(res.results[c]["out"]).reshape(BC, 1) for c in range(N_CORES)],
        axis=0,
    ).astype(np.float32)
    return out, res


def kernel(**inputs):
    out, _ = _run(inputs, trace=False)
    return out

